# revision 1
# baseline (speedup 1.0000x reference)
"""MoE (top-2 of 6 experts, dense-expert reference semantics) on 8 TRN2 cores.

Strategy: data-parallel over tokens (8192 tokens -> 1024/core), experts
replicated. Per core:
  - gating in fp32 on the tensor engine (top-2 margins are ~1e-5, bf16 would
    flip selections), top-2 + softmax via vector/scalar engines,
  - per-expert MLP in bf16 (x^T layout, [feature, token]): h^T = W1^T @ x^T,
    gelu(+b1) on ACT, combine-weight fold into h^T on DVE, then the second
    matmul accumulates all experts' contributions plus the combine@b2 term.
  - output written [D, T] per core; host transposes and concatenates.
No collectives needed.
"""

import sys

sys.path.insert(0, "/opt/trn_rl_repo")

import numpy as np
import ml_dtypes

import concourse.bass as bass  # noqa: F401  (registers engine classes)
import concourse.bacc as bacc
import concourse.mybir as mybir
from concourse import tile
from concourse import bass_utils

AF = mybir.ActivationFunctionType
ALU = mybir.AluOpType
AX = mybir.AxisListType
BF16 = mybir.dt.bfloat16
F32 = mybir.dt.float32

N_CORES = 8
B, S, D, E, H = 4, 2048, 1024, 6, 2048
TOKENS = B * S
T = TOKENS // N_CORES  # 1024 tokens per core
TC = 512               # token chunk = matmul moving free dim
NCH = T // TC          # 2 chunks
DB = D // 128          # 8 d blocks
JB = H // 128          # 16 hidden blocks
TB = T // 128          # 8 token blocks (gating)
NEG_BIG = -1.0e30


def _build_program():
    nc = bacc.Bacc("TRN2", target_bir_lowering=False, debug=False,
                   num_devices=N_CORES)

    xt_f = nc.dram_tensor("xt_f", [D, T], F32, kind="ExternalInput").ap()
    xt_b = nc.dram_tensor("xt_b", [D, T], BF16, kind="ExternalInput").ap()
    w1 = nc.dram_tensor("w1", [E, D, H], BF16, kind="ExternalInput").ap()
    w2 = nc.dram_tensor("w2", [E, H, D], BF16, kind="ExternalInput").ap()
    wg = nc.dram_tensor("wg", [D, E], F32, kind="ExternalInput").ap()
    bg = nc.dram_tensor("bg", [1, E], F32, kind="ExternalInput").ap()
    b1r = nc.dram_tensor("b1r", [128, E * JB], F32, kind="ExternalInput").ap()
    b2 = nc.dram_tensor("b2", [E, D], BF16, kind="ExternalInput").ap()
    ones = nc.dram_tensor("ones", [1, 128], F32, kind="ExternalInput").ap()
    sel = nc.dram_tensor("sel", [E, E * 128], BF16, kind="ExternalInput").ap()
    eye = nc.dram_tensor("eye", [128, 128], F32, kind="ExternalInput").ap()
    out = nc.dram_tensor("out", [D, T], F32, kind="ExternalOutput").ap()

    with tile.TileContext(nc) as tc:
        with (
            tc.tile_pool(name="constp", bufs=1) as constp,
            tc.tile_pool(name="xtfp", bufs=4) as xtfp,
            tc.tile_pool(name="xtbp", bufs=DB) as xtbp,
            tc.tile_pool(name="w1p", bufs=10) as w1p,
            tc.tile_pool(name="w2p", bufs=18) as w2p,
            tc.tile_pool(name="htp", bufs=18) as htp,
            tc.tile_pool(name="yaccp", bufs=DB) as yaccp,
            tc.tile_pool(name="crepp", bufs=E * NCH) as crepp,
            tc.tile_pool(name="gatp", bufs=2) as gatp,
            tc.tile_pool(name="psA", bufs=3, space="PSUM") as psA,
            tc.tile_pool(name="psB", bufs=3, space="PSUM") as psB,
        ):
            # ---- constants ----
            eye_sb = constp.tile([128, 128], F32, name="eye_sb", tag="eye")
            nc.sync.dma_start(eye_sb[:], eye[:])
            ones_sb = constp.tile([1, 128], F32, name="ones_sb", tag="ones")
            nc.sync.dma_start(ones_sb[:], ones[:])
            bg_sb = constp.tile([1, E], F32, name="bg_sb", tag="bg")
            nc.sync.dma_start(bg_sb[:], bg[:])
            sel_sb = constp.tile([E, E * 128], BF16, name="sel_sb", tag="sel")
            nc.sync.dma_start(sel_sb[:], sel[:])
            b1_sb = constp.tile([128, E * JB], F32, name="b1_sb", tag="b1")
            nc.sync.dma_start(b1_sb[:], b1r[:])
            b2_sb = constp.tile([E, D], BF16, name="b2_sb", tag="b2")
            nc.sync.dma_start(b2_sb[:], b2[:])
            wg_sb = []
            for d in range(DB):
                wgt = constp.tile([128, E], F32, name=f"wg_sb{d}", tag=f"wg{d}")
                nc.sync.dma_start(wgt[:], wg[d * 128:(d + 1) * 128, :])
                wg_sb.append(wgt)
            combT = constp.tile([E, T], BF16, name="combT", tag="combT")

            # ---- resident bf16 x^T ----
            xtb = []
            for d in range(DB):
                xt = xtbp.tile([128, T], BF16, name=f"xtb{d}", tag="xtb")
                nc.sync.dma_start(xt[:], xt_b[d * 128:(d + 1) * 128, :])
                xtb.append(xt)

            # ---- gating (fp32) ----
            for tb in range(TB):
                ps_g = psA.tile([128, E], F32, name="ps_g", tag="psA")
                for d in range(DB):
                    xg = xtfp.tile([128, 128], F32, name="xg", tag="xg")
                    nc.sync.dma_start(
                        xg[:], xt_f[d * 128:(d + 1) * 128,
                                    tb * 128:(tb + 1) * 128])
                    nc.tensor.matmul(ps_g[:], xg[:], wg_sb[d][:],
                                     start=(d == 0), stop=False)
                nc.tensor.matmul(ps_g[:], ones_sb[:], bg_sb[:],
                                 start=False, stop=True)
                lg = gatp.tile([128, E], F32, name="lg", tag="lg")
                nc.vector.tensor_copy(lg[:], ps_g[:])
                m1 = gatp.tile([128, 1], F32, name="m1", tag="m1")
                nc.vector.reduce_max(m1[:], lg[:], axis=AX.X)
                eq1 = gatp.tile([128, E], F32, name="eq1", tag="eq1")
                nc.vector.tensor_scalar(eq1[:], lg[:], m1[:], None,
                                        ALU.is_equal)
                mk = gatp.tile([128, E], F32, name="mk", tag="mk")
                nc.vector.scalar_tensor_tensor(mk[:], eq1[:], NEG_BIG, lg[:],
                                               ALU.mult, ALU.add)
                m2 = gatp.tile([128, 1], F32, name="m2", tag="m2")
                nc.vector.reduce_max(m2[:], mk[:], axis=AX.X)
                eq2 = gatp.tile([128, E], F32, name="eq2", tag="eq2")
                nc.vector.tensor_scalar(eq2[:], mk[:], m2[:], None,
                                        ALU.is_equal)
                dd = gatp.tile([128, 1], F32, name="dd", tag="dd")
                nc.vector.tensor_sub(dd[:], m2[:], m1[:])
                w2s = gatp.tile([128, 1], F32, name="w2s", tag="w2s")
                nc.scalar.activation(w2s[:], dd[:], AF.Sigmoid)
                w1s = gatp.tile([128, 1], F32, name="w1s", tag="w1s")
                nc.vector.tensor_scalar(w1s[:], w2s[:], -1.0, 1.0,
                                        ALU.mult, ALU.add)
                cb1 = gatp.tile([128, E], F32, name="cb1", tag="cb1")
                nc.vector.tensor_scalar(cb1[:], eq1[:], w1s[:], None, ALU.mult)
                cmb = gatp.tile([128, E], F32, name="cmb", tag="cmb")
                nc.vector.scalar_tensor_tensor(cmb[:], eq2[:], w2s[:], cb1[:],
                                               ALU.mult, ALU.add)
                ps_t = psA.tile([E, 128], F32, name="ps_t", tag="psA")
                nc.tensor.transpose(ps_t[:], cmb[:], eye_sb[:])
                nc.vector.tensor_copy(combT[:, tb * 128:(tb + 1) * 128],
                                      ps_t[:])

            # ---- combine weights broadcast across partitions (bf16) ----
            crep = [[None] * NCH for _ in range(E)]
            for e in range(E):
                for c in range(NCH):
                    ps_c = psA.tile([128, TC], F32, name="ps_c", tag="psA")
                    nc.tensor.matmul(ps_c[:],
                                     sel_sb[:, e * 128:(e + 1) * 128],
                                     combT[:, c * TC:(c + 1) * TC],
                                     start=True, stop=True)
                    cr = crepp.tile([128, TC], BF16, name=f"crep{e}_{c}",
                                    tag="crep")
                    nc.vector.tensor_copy(cr[:], ps_c[:])
                    crep[e][c] = cr

            # ---- expert loop ----
            yacc = []
            for d in range(DB):
                ya = yaccp.tile([128, T], F32, name=f"yacc{d}", tag="yacc")
                yacc.append(ya)

            for e in range(E):
                w1t = []
                for d in range(DB):
                    wt = w1p.tile([128, H], BF16, name=f"w1t{e}_{d}", tag="w1")
                    nc.sync.dma_start(wt[:], w1[e, d * 128:(d + 1) * 128, :])
                    w1t.append(wt)
                w2t = []
                for j in range(JB):
                    wt = w2p.tile([128, D], BF16, name=f"w2t{e}_{j}", tag="w2")
                    nc.sync.dma_start(wt[:], w2[e, j * 128:(j + 1) * 128, :])
                    w2t.append(wt)

                for c in range(NCH):
                    ht_list = []
                    for j in range(JB):
                        ps1 = psA.tile([128, TC], F32, name="ps1", tag="psA")
                        for d in range(DB):
                            nc.tensor.matmul(
                                ps1[:],
                                w1t[d][:, j * 128:(j + 1) * 128],
                                xtb[d][:, c * TC:(c + 1) * TC],
                                start=(d == 0), stop=(d == DB - 1))
                        ht = htp.tile([128, TC], BF16, name="ht", tag="ht")
                        nc.scalar.activation(
                            ht[:], ps1[:], AF.Gelu,
                            bias=b1_sb[:, e * JB + j:e * JB + j + 1])
                        nc.vector.tensor_tensor(ht[:], ht[:], crep[e][c][:],
                                                ALU.mult)
                        ht_list.append(ht)
                    for d2 in range(DB):
                        ps2 = psB.tile([128, TC], F32, name="ps2", tag="psB")
                        if e == 0:
                            nc.tensor.matmul(
                                ps2[:], b2_sb[:, d2 * 128:(d2 + 1) * 128],
                                combT[:, c * TC:(c + 1) * TC],
                                start=True, stop=False)
                        for j2 in range(JB):
                            nc.tensor.matmul(
                                ps2[:],
                                w2t[j2][:, d2 * 128:(d2 + 1) * 128],
                                ht_list[j2][:],
                                start=(e != 0 and j2 == 0),
                                stop=(j2 == JB - 1))
                        ysl = yacc[d2][:, c * TC:(c + 1) * TC]
                        if e == 0:
                            nc.vector.tensor_copy(ysl, ps2[:])
                        else:
                            nc.vector.tensor_tensor(ysl, ysl, ps2[:], ALU.add)

            for d in range(DB):
                nc.sync.dma_start(out[d * 128:(d + 1) * 128, :], yacc[d][:])

    nc.compile()
    return nc


_PROG = None


def _get_program():
    global _PROG
    if _PROG is None:
        _PROG = _build_program()
    return _PROG


def kernel(x, Wg, bg, W1, b1, W2, b2):
    nc = _get_program()

    xf = np.ascontiguousarray(x.reshape(TOKENS, D).astype(np.float32))
    W1b = np.ascontiguousarray(W1.astype(ml_dtypes.bfloat16))
    W2b = np.ascontiguousarray(W2.astype(ml_dtypes.bfloat16))
    b2b = np.ascontiguousarray(b2.astype(ml_dtypes.bfloat16))
    b1r = np.ascontiguousarray(
        b1.reshape(E, JB, 128).transpose(2, 0, 1).reshape(128, E * JB)
    ).astype(np.float32)
    ones_f = np.ones((1, 128), np.float32)
    eye_f = np.eye(128, dtype=np.float32)
    sel_b = np.zeros((E, E * 128), ml_dtypes.bfloat16)
    for e in range(E):
        sel_b[e, e * 128:(e + 1) * 128] = 1.0

    in_maps = []
    for c in range(N_CORES):
        xt = np.ascontiguousarray(xf[c * T:(c + 1) * T].T)  # [D, T] fp32
        in_maps.append({
            "xt_f": xt,
            "xt_b": xt.astype(ml_dtypes.bfloat16),
            "w1": W1b,
            "w2": W2b,
            "wg": np.ascontiguousarray(Wg.astype(np.float32)),
            "bg": np.ascontiguousarray(bg.astype(np.float32)).reshape(1, E),
            "b1r": b1r,
            "b2": b2b,
            "ones": ones_f,
            "sel": sel_b,
            "eye": eye_f,
        })

    res = bass_utils.run_bass_kernel_spmd(nc, in_maps,
                                          core_ids=list(range(N_CORES)))
    parts = [res.results[c]["out"].T for c in range(N_CORES)]  # [T, D] each
    return np.concatenate(parts, axis=0).reshape(B, S, D).astype(np.float32)
